# revision 4
# baseline (speedup 1.0000x reference)
"""Trainium2 Bass kernel for nn_AttentionResidualBlock.

Computation (per token t, head h):
    q = x @ W_q + b_q                     # [B,S,D]
    scores[t,h,l] = <q[t,h,:], k[t,l,h,:]> / sqrt(hd)
    w = softmax_l(scores)
    out[t,h,:] = sum_l w[t,h,l] * k[t,l,h,:]

Sharding: data-parallel over the 8192 (b,s) tokens -> 8 cores x 1024 tokens.

Per-core layout: token-major.  Each 128-token tile:
  - layer_history loaded as bf16 via SWDGE cast-DMA (HBM traffic unchanged,
    halves SBUF + enables DVE 2x tensor_tensor mode)
  - q_proj on PE in fp32r (full-rate, ~fp32 precision): x-tile transposed via
    PE transpose, then 16 accumulating matmuls + a k=1 "ones" matmul for b_q
  - scores: DVE bf16 mul (q broadcast over l via step-0 AP) + fold-tree
    reduce over hd (bf16 folds 64->8, fp32 tail reduce)
  - softmax over l=12 without max-subtraction (scores ~ N(0,1), exp is safe)
  - weights expanded over hd by exponential doubling (DVE+ACT), then DVE bf16
    mul and fold-tree sum over l (fp32 tail)
"""

import math
from contextlib import ExitStack

import numpy as np

import concourse.bass as bass
import concourse.tile as tile
from concourse import bacc, mybir
from concourse.bass_utils import run_bass_kernel_spmd
from concourse import masks

FP32 = mybir.dt.float32
FP32R = mybir.dt.float32r
BF16 = mybir.dt.bfloat16

B, S, L, D, H = 4, 2048, 12, 1024, 16
HD = D // H
N_CORES = 8
T = B * S // N_CORES          # tokens per core = 1024
P = 128                       # partition tile
NT = T // P                   # 8 token tiles per core
SCALE = 1.0 / math.sqrt(HD)   # 0.125


def _f32r(ap):
    return ap.bitcast(FP32R)


def build_body(ctx, tc, out, x, kh, wq, bq, ones):
    nc = tc.nc

    const_pool = ctx.enter_context(tc.tile_pool(name="const", bufs=1))
    # W as lhsT chunks: w_sb[p, c, j] = W[c*128 + p, j]
    w_sb = const_pool.tile([P, 8, D], FP32R)
    nc.sync.dma_start(w_sb[:], wq.rearrange("(c p) j -> p c j", p=P).bitcast(FP32R))
    bq_sb = const_pool.tile([1, D], FP32R)
    nc.sync.dma_start(bq_sb[:], bq.unsqueeze(0).bitcast(FP32R))
    ones_sb = const_pool.tile([1, P], FP32R)
    nc.sync.dma_start(ones_sb[:], ones.unsqueeze(0).bitcast(FP32R))
    ident = const_pool.tile([P, P], FP32)
    masks.make_identity(nc, ident[:])

    kp = ctx.enter_context(tc.tile_pool(name="k", bufs=2))
    xp = ctx.enter_context(tc.tile_pool(name="x", bufs=2))
    xtp = ctx.enter_context(tc.tile_pool(name="xt", bufs=2))
    qp = ctx.enter_context(tc.tile_pool(name="q", bufs=2))
    prodp = ctx.enter_context(tc.tile_pool(name="prod", bufs=2))
    wbp = ctx.enter_context(tc.tile_pool(name="wb", bufs=1))
    foldp = ctx.enter_context(tc.tile_pool(name="fold", bufs=1))
    sp = ctx.enter_context(tc.tile_pool(name="smx", bufs=2))
    outp = ctx.enter_context(tc.tile_pool(name="out", bufs=2))
    ps_t = ctx.enter_context(tc.tile_pool(name="ps_t", bufs=2, space="PSUM"))
    ps_q = ctx.enter_context(tc.tile_pool(name="ps_q", bufs=2, space="PSUM"))

    for tt in range(NT):
        tok = slice(tt * P, (tt + 1) * P)

        # ---- loads ----
        k_bf = kp.tile([P, L, D], BF16, tag="k")
        nc.gpsimd.dma_start(k_bf[:], kh[tok])  # fp32 -> bf16 cast DMA
        x_sb = xp.tile([P, D], FP32, tag="x")
        nc.sync.dma_start(x_sb[:], x[tok])

        # ---- transpose x tile: xt[p, c, t] = x[tok][t, c*128+p] ----
        xt_sb = xtp.tile([P, 8, P], FP32R, tag="xt")
        for c in range(8):
            xt_ps = ps_t.tile([P, P], FP32, tag="xtps")
            nc.tensor.transpose(xt_ps[:], x_sb[:, c * P:(c + 1) * P], ident[:])
            nc.scalar.copy(xt_sb[:, c, :], xt_ps[:])

        # ---- q = x @ W + b (token-major PSUM [t, d_out]) ----
        q_ps = ps_q.tile([P, D], FP32, tag="qps")
        for half in range(2):
            n0 = half * 512
            for c in range(8):
                nc.tensor.matmul(
                    q_ps[:, n0:n0 + 512],
                    lhsT=xt_sb[:, c, :],
                    rhs=w_sb[:, c, n0:n0 + 512],
                    start=(c == 0),
                    stop=False,
                )
            nc.tensor.matmul(
                q_ps[:, n0:n0 + 512],
                lhsT=ones_sb[:],
                rhs=bq_sb[:, n0:n0 + 512],
                start=False,
                stop=True,
            )
        # q -> SBUF bf16, folding in 1/sqrt(hd)
        q_bf = qp.tile([P, D], BF16, tag="q")
        nc.scalar.mul(q_bf[:], q_ps[:], SCALE)

        # ---- scores: prod = k * q (broadcast over l), fold-reduce over hd ----
        k4 = k_bf[:].rearrange("p l (h e) -> p l h e", h=H)
        qv = (
            q_bf[:]
            .rearrange("p (h e) -> p h e", h=H)
            .unsqueeze(1)
            .broadcast_to([P, L, H, HD])
        )
        prod = prodp.tile([P, L, H, HD], BF16, tag="prod")
        nc.vector.tensor_mul(prod[:], k4, qv)

        f1 = wbp.tile([P, L, H, 32], BF16, tag="wb")
        nc.vector.tensor_add(f1[:], prod[:, :, :, 0:32], prod[:, :, :, 32:64])
        f2 = foldp.tile([P, L, H, 16], BF16, tag="f2")
        nc.vector.tensor_add(f2[:], f1[:, :, :, 0:16], f1[:, :, :, 16:32])
        f3 = foldp.tile([P, L, H, 8], BF16, tag="f3")
        nc.vector.tensor_add(f3[:], f2[:, :, :, 0:8], f2[:, :, :, 8:16])
        scr = sp.tile([P, L, H], FP32, tag="scr")
        nc.vector.tensor_reduce(
            scr[:], f3[:], axis=mybir.AxisListType.X, op=mybir.AluOpType.add
        )

        # ---- softmax over l (no max subtraction) ----
        es = sp.tile([P, L, H], FP32, tag="es")
        nc.scalar.activation(es[:], scr[:], mybir.ActivationFunctionType.Exp)
        den = sp.tile([P, H], FP32, tag="den")
        nc.vector.tensor_reduce(
            den[:],
            es[:].rearrange("p l h -> p h l"),
            axis=mybir.AxisListType.X,
            op=mybir.AluOpType.add,
        )
        rd = sp.tile([P, H], FP32, tag="rd")
        nc.vector.reciprocal(rd[:], den[:])

        # normalized weights into slot 0 of the expanded tile
        wb = wbp.tile([P, L, H, HD], BF16, tag="wb")
        rdv = rd[:].unsqueeze(1).broadcast_to([P, L, H]).unsqueeze(3)
        nc.vector.tensor_mul(wb[:, :, :, 0:1], es[:].unsqueeze(3), rdv)
        # exponential-doubling broadcast over hd: 1->2->4->...->64
        for i in range(6):
            w0 = 1 << i
            src = wb[:, :, :, 0:w0]
            dst = wb[:, :, :, w0:2 * w0]
            if i < 3:
                nc.vector.tensor_copy(dst, src)
            else:
                nc.scalar.copy(dst, src)

        # ---- weighted sum over l ----
        prod2 = prodp.tile([P, L, D], BF16, tag="prod")
        nc.vector.tensor_mul(
            prod2[:], k_bf[:], wb[:].rearrange("p l h e -> p l (h e)")
        )
        s1 = wbp.tile([P, 6, D], BF16, tag="wb")
        nc.vector.tensor_add(s1[:], prod2[:, 0:6, :], prod2[:, 6:12, :])
        s2 = foldp.tile([P, 3, D], BF16, tag="f2")
        nc.vector.tensor_add(s2[:], s1[:, 0:3, :], s1[:, 3:6, :])
        s3 = foldp.tile([P, D], BF16, tag="f3")
        nc.vector.tensor_add(s3[:], s2[:, 0, :], s2[:, 1, :])
        o_sb = outp.tile([P, D], FP32, tag="o")
        nc.vector.tensor_add(o_sb[:], s3[:], s2[:, 2, :])

        nc.sync.dma_start(out[tok], o_sb[:])


_NC_CACHE = None


def build_nc():
    global _NC_CACHE
    if _NC_CACHE is not None:
        return _NC_CACHE
    nc = bacc.Bacc("TRN2", target_bir_lowering=False, debug=False,
                   num_devices=N_CORES)
    x = nc.dram_tensor("x", [T, D], FP32, kind="ExternalInput").ap()
    kh = nc.dram_tensor("kh", [T, L, D], FP32, kind="ExternalInput").ap()
    wq = nc.dram_tensor("wq", [D, D], FP32, kind="ExternalInput").ap()
    bq = nc.dram_tensor("bq", [D], FP32, kind="ExternalInput").ap()
    ones = nc.dram_tensor("ones", [P], FP32, kind="ExternalInput").ap()
    out = nc.dram_tensor("out", [T, D], FP32, kind="ExternalOutput").ap()
    with tile.TileContext(nc) as tc, ExitStack() as ctx:
        build_body(ctx, tc, out, x, kh, wq, bq, ones)
    nc.compile()
    _NC_CACHE = nc
    return nc


def make_in_maps(x_current, layer_history, W_q, b_q):
    x_flat = np.ascontiguousarray(
        x_current.reshape(B * S, D), dtype=np.float32)
    k_flat = np.ascontiguousarray(
        layer_history.reshape(B * S, L, D), dtype=np.float32)
    W_q = np.ascontiguousarray(W_q, dtype=np.float32)
    b_q = np.ascontiguousarray(b_q, dtype=np.float32)
    in_maps = []
    for c in range(N_CORES):
        sl = slice(c * T, (c + 1) * T)
        in_maps.append({
            "x": x_flat[sl],
            "kh": k_flat[sl],
            "wq": W_q,
            "bq": b_q,
            "ones": np.ones((P,), np.float32),
        })
    return in_maps


def kernel(x_current, layer_history, W_q, b_q):
    nc = build_nc()
    in_maps = make_in_maps(x_current, layer_history, W_q, b_q)
    res = run_bass_kernel_spmd(nc, in_maps, core_ids=list(range(N_CORES)))
    out = np.concatenate([res.results[c]["out"] for c in range(N_CORES)], axis=0)
    return out.reshape(B, S, D).astype(np.float32)


if __name__ == "__main__":
    rng = np.random.default_rng(0)
    x = rng.standard_normal((B, S, D), dtype=np.float32)
    k = rng.standard_normal((B, S, L, D), dtype=np.float32)
    W = (rng.standard_normal((D, D), dtype=np.float32) / math.sqrt(D)).astype(np.float32)
    b = (rng.standard_normal((D,), dtype=np.float32) * 0.01).astype(np.float32)
    o = kernel(x, k, W, b)
    print("ok", o.shape, o.dtype, float(np.abs(o).mean()))


# revision 5
# speedup vs baseline: 9.6142x; 9.6142x over previous
"""Trainium2 Bass kernel for nn_AttentionResidualBlock.

Computation (per token t, head h):
    q = x @ W_q + b_q                     # [B,S,D]
    scores[t,h,l] = <q[t,h,:], k[t,l,h,:]> / sqrt(hd)
    w = softmax_l(scores)
    out[t,h,:] = sum_l w[t,h,l] * k[t,l,h,:]

Sharding: data-parallel over the 8192 (b,s) tokens -> 8 cores x 1024 tokens.

Per-core layout: token-major.  Each 128-token tile:
  - layer_history loaded as bf16 via SWDGE cast-DMA (HBM traffic unchanged,
    halves SBUF + enables DVE 2x tensor_tensor mode)
  - q_proj on PE in fp32r (full-rate, ~fp32 precision): x-tile transposed via
    PE transpose, then 16 accumulating matmuls + a k=1 "ones" matmul for b_q
  - scores: DVE bf16 mul (q broadcast over l via step-0 AP) + fold-tree
    reduce over hd (bf16 folds 64->8, fp32 tail reduce)
  - softmax over l=12 without max-subtraction (scores ~ N(0,1), exp is safe)
  - weights expanded over hd by exponential doubling (DVE+ACT), then DVE bf16
    mul and fold-tree sum over l (fp32 tail)
"""

import math
from contextlib import ExitStack

import numpy as np

import concourse.bass as bass
import concourse.tile as tile
from concourse import bacc, mybir
from concourse.bass_utils import run_bass_kernel_spmd
from concourse import masks

FP32 = mybir.dt.float32
FP32R = mybir.dt.float32r
BF16 = mybir.dt.bfloat16

B, S, L, D, H = 4, 2048, 12, 1024, 16
HD = D // H
N_CORES = 8
T = B * S // N_CORES          # tokens per core = 1024
P = 128                       # partition tile
NT = T // P                   # 8 token tiles per core
SCALE = 1.0 / math.sqrt(HD)   # 0.125


def _f32r(ap):
    return ap.bitcast(FP32R)


def build_body(ctx, tc, out, x, kh, wq, bq, ones, repeat=1):
    nc = tc.nc

    const_pool = ctx.enter_context(tc.tile_pool(name="const", bufs=1))
    # W as lhsT chunks: w_sb[p, c, j] = W[c*128 + p, j]
    w_sb = const_pool.tile([P, 8, D], FP32R)
    nc.sync.dma_start(w_sb[:], wq.rearrange("(c p) j -> p c j", p=P).bitcast(FP32R))
    bq_sb = const_pool.tile([1, D], FP32R)
    nc.sync.dma_start(bq_sb[:], bq.unsqueeze(0).bitcast(FP32R))
    ones_sb = const_pool.tile([1, P], FP32R)
    nc.sync.dma_start(ones_sb[:], ones.unsqueeze(0).bitcast(FP32R))
    ident = const_pool.tile([P, P], FP32)
    masks.make_identity(nc, ident[:])

    kp = ctx.enter_context(tc.tile_pool(name="k", bufs=2))
    xp = ctx.enter_context(tc.tile_pool(name="x", bufs=2))
    xtp = ctx.enter_context(tc.tile_pool(name="xt", bufs=2))
    qp = ctx.enter_context(tc.tile_pool(name="q", bufs=2))
    prodp = ctx.enter_context(tc.tile_pool(name="prod", bufs=2))
    wbp = ctx.enter_context(tc.tile_pool(name="wb", bufs=1))
    foldp = ctx.enter_context(tc.tile_pool(name="fold", bufs=1))
    sp = ctx.enter_context(tc.tile_pool(name="smx", bufs=2))
    outp = ctx.enter_context(tc.tile_pool(name="out", bufs=2))
    ps_t = ctx.enter_context(tc.tile_pool(name="ps_t", bufs=2, space="PSUM"))
    ps_q = ctx.enter_context(tc.tile_pool(name="ps_q", bufs=2, space="PSUM"))

    for tt in range(NT * repeat):
        tt = tt % NT
        tok = slice(tt * P, (tt + 1) * P)

        # ---- loads ----
        k_bf = kp.tile([P, L, D], BF16, tag="k")
        nc.gpsimd.dma_start(k_bf[:], kh[tok])  # fp32 -> bf16 cast DMA
        x_sb = xp.tile([P, D], FP32, tag="x")
        nc.sync.dma_start(x_sb[:], x[tok])

        # ---- transpose x tile: xt[p, c, t] = x[tok][t, c*128+p] ----
        xt_sb = xtp.tile([P, 8, P], FP32R, tag="xt")
        for c in range(8):
            xt_ps = ps_t.tile([P, P], FP32, tag="xtps")
            nc.tensor.transpose(xt_ps[:], x_sb[:, c * P:(c + 1) * P], ident[:])
            nc.scalar.copy(xt_sb[:, c, :], xt_ps[:])

        # ---- q = x @ W + b (token-major PSUM [t, d_out]) ----
        q_ps = ps_q.tile([P, D], FP32, tag="qps")
        for half in range(2):
            n0 = half * 512
            for c in range(8):
                nc.tensor.matmul(
                    q_ps[:, n0:n0 + 512],
                    lhsT=xt_sb[:, c, :],
                    rhs=w_sb[:, c, n0:n0 + 512],
                    start=(c == 0),
                    stop=False,
                )
            nc.tensor.matmul(
                q_ps[:, n0:n0 + 512],
                lhsT=ones_sb[:],
                rhs=bq_sb[:, n0:n0 + 512],
                start=False,
                stop=True,
            )
        # q -> SBUF bf16, folding in 1/sqrt(hd)
        q_bf = qp.tile([P, D], BF16, tag="q")
        nc.scalar.mul(q_bf[:], q_ps[:], SCALE)

        # ---- scores: prod = k * q (broadcast over l), fold-reduce over hd ----
        k4 = k_bf[:].rearrange("p l (h e) -> p l h e", h=H)
        qv = (
            q_bf[:]
            .rearrange("p (h e) -> p h e", h=H)
            .unsqueeze(1)
            .broadcast_to([P, L, H, HD])
        )
        prod = prodp.tile([P, L, H, HD], BF16, tag="prod")
        nc.vector.tensor_mul(prod[:], k4, qv)

        f1 = wbp.tile([P, L, H, 32], BF16, tag="wb")
        nc.vector.tensor_add(f1[:], prod[:, :, :, 0:32], prod[:, :, :, 32:64])
        f2 = foldp.tile([P, L, H, 16], BF16, tag="f2")
        nc.vector.tensor_add(f2[:], f1[:, :, :, 0:16], f1[:, :, :, 16:32])
        f3 = foldp.tile([P, L, H, 8], BF16, tag="f3")
        nc.vector.tensor_add(f3[:], f2[:, :, :, 0:8], f2[:, :, :, 8:16])
        scr = sp.tile([P, L, H], FP32, tag="scr")
        nc.vector.tensor_reduce(
            scr[:], f3[:], axis=mybir.AxisListType.X, op=mybir.AluOpType.add
        )

        # ---- softmax over l (no max subtraction) ----
        es = sp.tile([P, L, H], FP32, tag="es")
        nc.scalar.activation(es[:], scr[:], mybir.ActivationFunctionType.Exp)
        den = sp.tile([P, H], FP32, tag="den")
        nc.vector.tensor_reduce(
            den[:],
            es[:].rearrange("p l h -> p h l"),
            axis=mybir.AxisListType.X,
            op=mybir.AluOpType.add,
        )
        rd = sp.tile([P, H], FP32, tag="rd")
        nc.vector.reciprocal(rd[:], den[:])

        # normalized weights into slot 0 of the expanded tile
        wb = wbp.tile([P, L, H, HD], BF16, tag="wb")
        rdv = rd[:].unsqueeze(1).broadcast_to([P, L, H]).unsqueeze(3)
        nc.vector.tensor_mul(wb[:, :, :, 0:1], es[:].unsqueeze(3), rdv)
        # exponential-doubling broadcast over hd: 1->2->4->...->64
        for i in range(6):
            w0 = 1 << i
            src = wb[:, :, :, 0:w0]
            dst = wb[:, :, :, w0:2 * w0]
            if i < 3:
                nc.vector.tensor_copy(dst, src)
            else:
                nc.scalar.copy(dst, src)

        # ---- weighted sum over l ----
        prod2 = prodp.tile([P, L, D], BF16, tag="prod")
        nc.vector.tensor_mul(
            prod2[:], k_bf[:], wb[:].rearrange("p l h e -> p l (h e)")
        )
        s1 = wbp.tile([P, 6, D], BF16, tag="wb")
        nc.vector.tensor_add(s1[:], prod2[:, 0:6, :], prod2[:, 6:12, :])
        s2 = foldp.tile([P, 3, D], BF16, tag="f2")
        nc.vector.tensor_add(s2[:], s1[:, 0:3, :], s1[:, 3:6, :])
        s3 = foldp.tile([P, D], BF16, tag="f3")
        nc.vector.tensor_add(s3[:], s2[:, 0, :], s2[:, 1, :])
        o_sb = outp.tile([P, D], FP32, tag="o")
        nc.vector.tensor_add(o_sb[:], s3[:], s2[:, 2, :])

        nc.sync.dma_start(out[tok], o_sb[:])


_NC_CACHE = {}


def build_nc(repeat=1):
    if repeat in _NC_CACHE:
        return _NC_CACHE[repeat]
    nc = bacc.Bacc("TRN2", target_bir_lowering=False, debug=False,
                   num_devices=N_CORES)
    x = nc.dram_tensor("x", [T, D], FP32, kind="ExternalInput").ap()
    kh = nc.dram_tensor("kh", [T, L, D], FP32, kind="ExternalInput").ap()
    wq = nc.dram_tensor("wq", [D, D], FP32, kind="ExternalInput").ap()
    bq = nc.dram_tensor("bq", [D], FP32, kind="ExternalInput").ap()
    ones = nc.dram_tensor("ones", [P], FP32, kind="ExternalInput").ap()
    out = nc.dram_tensor("out", [T, D], FP32, kind="ExternalOutput").ap()
    with tile.TileContext(nc) as tc, ExitStack() as ctx:
        build_body(ctx, tc, out, x, kh, wq, bq, ones, repeat=repeat)
    nc.compile()
    _NC_CACHE[repeat] = nc
    return nc


def make_in_maps(x_current, layer_history, W_q, b_q):
    x_flat = np.ascontiguousarray(
        x_current.reshape(B * S, D), dtype=np.float32)
    k_flat = np.ascontiguousarray(
        layer_history.reshape(B * S, L, D), dtype=np.float32)
    W_q = np.ascontiguousarray(W_q, dtype=np.float32)
    b_q = np.ascontiguousarray(b_q, dtype=np.float32)
    in_maps = []
    for c in range(N_CORES):
        sl = slice(c * T, (c + 1) * T)
        in_maps.append({
            "x": x_flat[sl],
            "kh": k_flat[sl],
            "wq": W_q,
            "bq": b_q,
            "ones": np.ones((P,), np.float32),
        })
    return in_maps


def kernel(x_current, layer_history, W_q, b_q):
    nc = build_nc()
    in_maps = make_in_maps(x_current, layer_history, W_q, b_q)
    res = run_bass_kernel_spmd(nc, in_maps, core_ids=list(range(N_CORES)))
    out = np.concatenate([res.results[c]["out"] for c in range(N_CORES)], axis=0)
    return out.reshape(B, S, D).astype(np.float32)


if __name__ == "__main__":
    rng = np.random.default_rng(0)
    x = rng.standard_normal((B, S, D), dtype=np.float32)
    k = rng.standard_normal((B, S, L, D), dtype=np.float32)
    W = (rng.standard_normal((D, D), dtype=np.float32) / math.sqrt(D)).astype(np.float32)
    b = (rng.standard_normal((D,), dtype=np.float32) * 0.01).astype(np.float32)
    o = kernel(x, k, W, b)
    print("ok", o.shape, o.dtype, float(np.abs(o).mean()))


# revision 8
# speedup vs baseline: 11.1406x; 1.1588x over previous
"""Trainium2 Bass kernel for nn_AttentionResidualBlock.

Computation (per token t, head h):
    q = x @ W_q + b_q                     # [B,S,D]
    scores[t,h,l] = <q[t,h,:], k[t,l,h,:]> / sqrt(hd)
    w = softmax_l(scores)
    out[t,h,:] = sum_l w[t,h,l] * k[t,l,h,:]

Sharding: data-parallel over the 8192 (b,s) tokens -> 8 cores x 1024 tokens.

Per-core layout: token-major.  Each 128-token tile:
  - layer_history loaded as bf16 via SWDGE cast-DMA (HBM traffic unchanged,
    halves SBUF + enables DVE 2x tensor_tensor mode)
  - q_proj on PE in fp32r (full-rate, ~fp32 precision): x-tile transposed via
    PE transpose, then 16 accumulating matmuls + a k=1 "ones" matmul for b_q
  - scores: DVE bf16 mul (q broadcast over l via step-0 AP) + fold-tree
    reduce over hd (bf16 folds 64->8, fp32 tail reduce)
  - softmax over l=12 without max-subtraction (scores ~ N(0,1), exp is safe)
  - weights expanded over hd by exponential doubling (DVE+ACT), then DVE bf16
    mul and fold-tree sum over l (fp32 tail)
"""

import math
from contextlib import ExitStack

import numpy as np

import concourse.bass as bass
import concourse.tile as tile
from concourse import bacc, mybir
from concourse.bass_utils import run_bass_kernel_spmd
from concourse import masks

FP32 = mybir.dt.float32
FP32R = mybir.dt.float32r
BF16 = mybir.dt.bfloat16

B, S, L, D, H = 4, 2048, 12, 1024, 16
HD = D // H
N_CORES = 8
T = B * S // N_CORES          # tokens per core = 1024
P = 128                       # partition tile
NT = T // P                   # 8 token tiles per core
SCALE = 1.0 / math.sqrt(HD)   # 0.125


def _f32r(ap):
    return ap.bitcast(FP32R)


def build_body(ctx, tc, out, x, kh, wq, bq, ones, repeat=1):
    nc = tc.nc

    const_pool = ctx.enter_context(tc.tile_pool(name="const", bufs=1))
    # W as lhsT chunks: w_sb[p, c, j] = W[c*128 + p, j]
    w_sb = const_pool.tile([P, 8, D], FP32R)
    nc.sync.dma_start(w_sb[:], wq.rearrange("(c p) j -> p c j", p=P).bitcast(FP32R))
    bq_sb = const_pool.tile([1, D], BF16)
    nc.gpsimd.dma_start(bq_sb[:], bq.unsqueeze(0))
    ones_sb = const_pool.tile([1, P], BF16)
    nc.gpsimd.dma_start(ones_sb[:], ones.unsqueeze(0))
    ident = const_pool.tile([P, P], FP32)
    masks.make_identity(nc, ident[:])

    kp = ctx.enter_context(tc.tile_pool(name="k", bufs=2))
    xp = ctx.enter_context(tc.tile_pool(name="x", bufs=2))
    xtp = ctx.enter_context(tc.tile_pool(name="xt", bufs=2))
    qp = ctx.enter_context(tc.tile_pool(name="q", bufs=2))
    prodp = ctx.enter_context(tc.tile_pool(name="prod", bufs=2))
    wbp = ctx.enter_context(tc.tile_pool(name="wb", bufs=2))
    sp = ctx.enter_context(tc.tile_pool(name="smx", bufs=1))
    outp = ctx.enter_context(tc.tile_pool(name="out", bufs=1))
    ps_t = ctx.enter_context(tc.tile_pool(name="ps_t", bufs=2, space="PSUM"))
    ps_q = ctx.enter_context(tc.tile_pool(name="ps_q", bufs=2, space="PSUM"))

    for tt in range(NT * repeat):
        tt = tt % NT
        tok = slice(tt * P, (tt + 1) * P)

        # ---- loads ----
        k_bf = kp.tile([P, L, D], BF16, tag="k")
        nc.gpsimd.dma_start(k_bf[:], kh[tok])  # fp32 -> bf16 cast DMA
        x_sb = xp.tile([P, D], FP32, tag="x")
        nc.sync.dma_start(x_sb[:], x[tok])

        # ---- transpose x tile: xt[p, c, t] = x[tok][t, c*128+p] ----
        xt_sb = xtp.tile([P, 8, P], FP32R, tag="xt")
        for c in range(8):
            xt_ps = ps_t.tile([P, P], FP32, tag="xtps")
            nc.tensor.transpose(xt_ps[:], x_sb[:, c * P:(c + 1) * P], ident[:])
            nc.scalar.copy(xt_sb[:, c, :], xt_ps[:])

        # ---- q = x @ W + b (token-major PSUM [t, d_out]) ----
        q_ps = ps_q.tile([P, D], FP32, tag="qps")
        for half in range(2):
            n0 = half * 512
            for c in range(8):
                nc.tensor.matmul(
                    q_ps[:, n0:n0 + 512],
                    lhsT=xt_sb[:, c, :],
                    rhs=w_sb[:, c, n0:n0 + 512],
                    start=(c == 0),
                    stop=False,
                )
            nc.tensor.matmul(
                q_ps[:, n0:n0 + 512],
                lhsT=ones_sb[:],
                rhs=bq_sb[:, n0:n0 + 512],
                start=False,
                stop=True,
            )
        # q -> SBUF bf16, folding in 1/sqrt(hd)
        q_bf = qp.tile([P, D], BF16, tag="q")
        nc.scalar.mul(q_bf[:], q_ps[:], SCALE)

        # ---- scores: prod = k * q (broadcast over l), fold-reduce over hd ----
        k4 = k_bf[:].rearrange("p l (h e) -> p l h e", h=H)
        qv = (
            q_bf[:]
            .rearrange("p (h e) -> p h e", h=H)
            .unsqueeze(1)
            .broadcast_to([P, L, H, HD])
        )
        prod = prodp.tile([P, L, H, HD], BF16, tag="prod")
        nc.vector.tensor_mul(prod[:], k4, qv)

        # in-place fold tree over hd: 64->32->16->8->4->2, then fp32 tail add.
        # dst aliases in1 exactly (same element positions) which is safe for
        # the streaming DVE.
        off = 0
        for w0 in (32, 16, 8, 4, 2):
            nc.vector.tensor_add(
                prod[:, :, :, off + w0:off + 2 * w0],
                prod[:, :, :, off:off + w0],
                prod[:, :, :, off + w0:off + 2 * w0],
            )
            off += w0
        # off == 62: two surviving partials at 62, 63
        scr = sp.tile([P, L, H], FP32, tag="scr")
        nc.vector.tensor_add(
            scr[:].unsqueeze(3), prod[:, :, :, 62:63], prod[:, :, :, 63:64]
        )

        # ---- softmax over l (no max subtraction) ----
        es = sp.tile([P, L, H], FP32, tag="es")
        nc.scalar.activation(es[:], scr[:], mybir.ActivationFunctionType.Exp)
        den = sp.tile([P, H], FP32, tag="den")
        nc.vector.tensor_reduce(
            den[:],
            es[:].rearrange("p l h -> p h l"),
            axis=mybir.AxisListType.X,
            op=mybir.AluOpType.add,
        )
        rd = sp.tile([P, H], FP32, tag="rd")
        nc.vector.reciprocal(rd[:], den[:])

        # normalized weights into slots 0,1 of the expanded tile (a bf16
        # pair = one fp32 word), then fp32-word doubling broadcast on ACT.
        wb = wbp.tile([P, L, H, HD], BF16, tag="wb")
        rdv = rd[:].unsqueeze(1).broadcast_to([P, L, H]).unsqueeze(3)
        nc.vector.tensor_mul(
            wb[:, :, :, 0:2],
            es[:].unsqueeze(3).broadcast_to([P, L, H, 2]),
            rdv.broadcast_to([P, L, H, 2]),
        )
        wbf = wb[:].bitcast(FP32)  # [P, L, H, 32] fp32 words (bf16 pairs)
        for i in range(5):
            w0 = 1 << i
            nc.scalar.copy(wbf[:, :, :, w0:2 * w0], wbf[:, :, :, 0:w0])

        # ---- weighted sum over l ----
        prod2 = prodp.tile([P, L, D], BF16, tag="prod")
        nc.vector.tensor_mul(
            prod2[:], k_bf[:], wb[:].rearrange("p l h e -> p l (h e)")
        )
        # in-place fold over l: 12->6->3, then 3->1 with fp32 tail
        nc.vector.tensor_add(prod2[:, 6:12, :], prod2[:, 0:6, :], prod2[:, 6:12, :])
        nc.vector.tensor_add(prod2[:, 9:12, :], prod2[:, 6:9, :], prod2[:, 9:12, :])
        nc.vector.tensor_add(prod2[:, 10, :], prod2[:, 9, :], prod2[:, 10, :])
        o_sb = outp.tile([P, D], FP32, tag="o")
        nc.vector.tensor_add(o_sb[:], prod2[:, 10, :], prod2[:, 11, :])

        nc.sync.dma_start(out[tok], o_sb[:])


_NC_CACHE = {}


def build_nc(repeat=1):
    if repeat in _NC_CACHE:
        return _NC_CACHE[repeat]
    nc = bacc.Bacc("TRN2", target_bir_lowering=False, debug=False,
                   num_devices=N_CORES)
    x = nc.dram_tensor("x", [T, D], FP32, kind="ExternalInput").ap()
    kh = nc.dram_tensor("kh", [T, L, D], FP32, kind="ExternalInput").ap()
    wq = nc.dram_tensor("wq", [D, D], FP32, kind="ExternalInput").ap()
    bq = nc.dram_tensor("bq", [D], FP32, kind="ExternalInput").ap()
    ones = nc.dram_tensor("ones", [P], FP32, kind="ExternalInput").ap()
    out = nc.dram_tensor("out", [T, D], FP32, kind="ExternalOutput").ap()
    with tile.TileContext(nc) as tc, ExitStack() as ctx:
        build_body(ctx, tc, out, x, kh, wq, bq, ones, repeat=repeat)
    nc.compile()
    _NC_CACHE[repeat] = nc
    return nc


def make_in_maps(x_current, layer_history, W_q, b_q):
    x_flat = np.ascontiguousarray(
        x_current.reshape(B * S, D), dtype=np.float32)
    k_flat = np.ascontiguousarray(
        layer_history.reshape(B * S, L, D), dtype=np.float32)
    W_q = np.ascontiguousarray(W_q, dtype=np.float32)
    b_q = np.ascontiguousarray(b_q, dtype=np.float32)
    in_maps = []
    for c in range(N_CORES):
        sl = slice(c * T, (c + 1) * T)
        in_maps.append({
            "x": x_flat[sl],
            "kh": k_flat[sl],
            "wq": W_q,
            "bq": b_q,
            "ones": np.ones((P,), np.float32),
        })
    return in_maps


def kernel(x_current, layer_history, W_q, b_q):
    nc = build_nc()
    in_maps = make_in_maps(x_current, layer_history, W_q, b_q)
    res = run_bass_kernel_spmd(nc, in_maps, core_ids=list(range(N_CORES)))
    out = np.concatenate([res.results[c]["out"] for c in range(N_CORES)], axis=0)
    return out.reshape(B, S, D).astype(np.float32)


if __name__ == "__main__":
    rng = np.random.default_rng(0)
    x = rng.standard_normal((B, S, D), dtype=np.float32)
    k = rng.standard_normal((B, S, L, D), dtype=np.float32)
    W = (rng.standard_normal((D, D), dtype=np.float32) / math.sqrt(D)).astype(np.float32)
    b = (rng.standard_normal((D,), dtype=np.float32) * 0.01).astype(np.float32)
    o = kernel(x, k, W, b)
    print("ok", o.shape, o.dtype, float(np.abs(o).mean()))


# revision 11
# speedup vs baseline: 13.2481x; 1.1892x over previous
"""Trainium2 Bass kernel for nn_AttentionResidualBlock.

Computation (per token t, head h):
    q = x @ W_q + b_q                     # [B,S,D]
    scores[t,h,l] = <q[t,h,:], k[t,l,h,:]> / sqrt(hd)
    w = softmax_l(scores)
    out[t,h,:] = sum_l w[t,h,l] * k[t,l,h,:]

Sharding: data-parallel over the 8192 (b,s) tokens -> 8 cores x 1024 tokens.

Per-core layout: token-major.  Each 128-token tile:
  - layer_history loaded as bf16 via SWDGE cast-DMA (HBM traffic unchanged,
    halves SBUF + enables DVE 2x tensor_tensor mode)
  - q_proj on PE in fp32r (full-rate, ~fp32 precision): x-tile transposed via
    PE transpose, then 16 accumulating matmuls + a k=1 "ones" matmul for b_q
  - scores: DVE bf16 mul (q broadcast over l via step-0 AP) + fold-tree
    reduce over hd (bf16 folds 64->8, fp32 tail reduce)
  - softmax over l=12 without max-subtraction (scores ~ N(0,1), exp is safe)
  - weights expanded over hd by exponential doubling (DVE+ACT), then DVE bf16
    mul and fold-tree sum over l (fp32 tail)
"""

import math
from contextlib import ExitStack

import numpy as np

import concourse.bass as bass
import concourse.tile as tile
from concourse import bacc, mybir
from concourse.bass_utils import run_bass_kernel_spmd
from concourse import masks

FP32 = mybir.dt.float32
FP32R = mybir.dt.float32r
BF16 = mybir.dt.bfloat16

B, S, L, D, H = 4, 2048, 12, 1024, 16
HD = D // H
N_CORES = 8
T = B * S // N_CORES          # tokens per core = 1024
P = 128                       # partition tile
NT = T // P                   # 8 token tiles per core
SCALE = 1.0 / math.sqrt(HD)   # 0.125


def _f32r(ap):
    return ap.bitcast(FP32R)


def build_body(ctx, tc, out, x, kh, wq, bq, ones, repeat=1):
    nc = tc.nc

    const_pool = ctx.enter_context(tc.tile_pool(name="const", bufs=1))
    # W as lhsT chunks: w_sb[p, c, j] = W[c*128 + p, j]
    w_sb = const_pool.tile([P, 8, D], FP32R)
    nc.scalar.dma_start(w_sb[:], wq.rearrange("(c p) j -> p c j", p=P).bitcast(FP32R))
    bq_sb = const_pool.tile([1, D], BF16)
    nc.gpsimd.dma_start(bq_sb[:], bq.unsqueeze(0))
    ones_sb = const_pool.tile([1, P], BF16)
    nc.gpsimd.dma_start(ones_sb[:], ones.unsqueeze(0))
    ident = const_pool.tile([P, P], FP32)
    masks.make_identity(nc, ident[:])

    kp = ctx.enter_context(tc.tile_pool(name="k", bufs=2))
    xp = ctx.enter_context(tc.tile_pool(name="x", bufs=2))
    xtp = ctx.enter_context(tc.tile_pool(name="xt", bufs=2))
    qp = ctx.enter_context(tc.tile_pool(name="q", bufs=2))
    prodp = ctx.enter_context(tc.tile_pool(name="prod", bufs=2))
    wbp = ctx.enter_context(tc.tile_pool(name="wb", bufs=2))
    sp = ctx.enter_context(tc.tile_pool(name="smx", bufs=1))
    outp = ctx.enter_context(tc.tile_pool(name="out", bufs=1))
    ps_t = ctx.enter_context(tc.tile_pool(name="ps_t", bufs=2, space="PSUM"))
    ps_q = ctx.enter_context(tc.tile_pool(name="ps_q", bufs=2, space="PSUM"))

    for tt in range(NT * repeat):
        tt = tt % NT
        tok = slice(tt * P, (tt + 1) * P)

        # ---- loads ----
        k_bf = kp.tile([P, L, D], BF16, tag="k")
        nc.gpsimd.dma_start(k_bf[:], kh[tok])  # fp32 -> bf16 cast DMA
        x_sb = xp.tile([P, D], FP32, tag="x")
        nc.sync.dma_start(x_sb[:], x[tok])

        # ---- transpose x tile: xt[p, c, t] = x[tok][t, c*128+p] ----
        xt_sb = xtp.tile([P, 8, P], FP32R, tag="xt")
        for c in range(8):
            xt_ps = ps_t.tile([P, P], FP32, tag="xtps")
            nc.tensor.transpose(xt_ps[:], x_sb[:, c * P:(c + 1) * P], ident[:])
            nc.scalar.copy(xt_sb[:, c, :], xt_ps[:])

        # ---- q = x @ W + b (token-major PSUM [t, d_out]) ----
        q_ps = ps_q.tile([P, D], FP32, tag="qps")
        for half in range(2):
            n0 = half * 512
            for c in range(8):
                nc.tensor.matmul(
                    q_ps[:, n0:n0 + 512],
                    lhsT=xt_sb[:, c, :],
                    rhs=w_sb[:, c, n0:n0 + 512],
                    start=(c == 0),
                    stop=False,
                )
            nc.tensor.matmul(
                q_ps[:, n0:n0 + 512],
                lhsT=ones_sb[:],
                rhs=bq_sb[:, n0:n0 + 512],
                start=False,
                stop=True,
            )
        # q -> SBUF bf16, folding in 1/sqrt(hd)
        q_bf = qp.tile([P, D], BF16, tag="q")
        nc.scalar.mul(q_bf[:], q_ps[:], SCALE)

        # ---- scores: prod = k * q (broadcast over l), fold-reduce over hd ----
        k4 = k_bf[:].rearrange("p l (h e) -> p l h e", h=H)
        qv = (
            q_bf[:]
            .rearrange("p (h e) -> p h e", h=H)
            .unsqueeze(1)
            .broadcast_to([P, L, H, HD])
        )
        prod = prodp.tile([P, L, H, HD], BF16, tag="prod")
        nc.vector.tensor_mul(prod[:], k4, qv)

        # in-place fold tree over hd: 64->32->16->8->4->2, then fp32 tail add.
        # dst aliases in1 exactly (same element positions) which is safe for
        # the streaming DVE.
        off = 0
        for w0 in (32, 16, 8, 4, 2):
            nc.vector.tensor_add(
                prod[:, :, :, off + w0:off + 2 * w0],
                prod[:, :, :, off:off + w0],
                prod[:, :, :, off + w0:off + 2 * w0],
            )
            off += w0
        # off == 62: two surviving partials at 62, 63
        scr = sp.tile([P, L, H], FP32, tag="scr")
        nc.vector.tensor_add(
            scr[:].unsqueeze(3), prod[:, :, :, 62:63], prod[:, :, :, 63:64]
        )

        # ---- softmax over l (no max subtraction) ----
        es = sp.tile([P, L, H], FP32, tag="es")
        nc.scalar.activation(es[:], scr[:], mybir.ActivationFunctionType.Exp)
        den = sp.tile([P, H], FP32, tag="den")
        nc.vector.tensor_reduce(
            den[:],
            es[:].rearrange("p l h -> p h l"),
            axis=mybir.AxisListType.X,
            op=mybir.AluOpType.add,
        )
        rd = sp.tile([P, H], FP32, tag="rd")
        nc.vector.reciprocal(rd[:], den[:])

        # normalized weights into slots 0,1 of the expanded tile (a bf16
        # pair = one fp32 word), then fp32-word doubling broadcast on ACT.
        wb = wbp.tile([P, L, H, HD], BF16, tag="wb")
        rdv = rd[:].unsqueeze(1).broadcast_to([P, L, H]).unsqueeze(3)
        nc.vector.tensor_mul(
            wb[:, :, :, 0:2],
            es[:].unsqueeze(3).broadcast_to([P, L, H, 2]),
            rdv.broadcast_to([P, L, H, 2]),
        )
        # expand each bf16 pair (one fp32 word) across hd with a single
        # step-0-source broadcast copy on ACT, split into l-halves so the
        # first half of the ws-mul overlaps the second half's expansion.
        wbf = wb[:].bitcast(FP32)  # [P, L, H, 32] fp32 words (bf16 pairs)
        prod2 = prodp.tile([P, L, D], BF16, tag="prod")
        wbflat = wb[:].rearrange("p l h e -> p l (h e)")
        for lh in range(2):
            ls = slice(lh * 6, (lh + 1) * 6)
            nc.scalar.copy(
                wbf[:, ls, :, 1:32],
                wbf[:, ls, :, 0:1].broadcast_to([P, 6, H, 31]),
            )
            nc.vector.tensor_mul(
                prod2[:, ls, :], k_bf[:, ls, :], wbflat[:, ls, :]
            )
        # in-place fold over l: 12->6->3, then 3->1 with fp32 tail
        nc.vector.tensor_add(prod2[:, 6:12, :], prod2[:, 0:6, :], prod2[:, 6:12, :])
        nc.vector.tensor_add(prod2[:, 9:12, :], prod2[:, 6:9, :], prod2[:, 9:12, :])
        nc.vector.tensor_add(prod2[:, 10, :], prod2[:, 9, :], prod2[:, 10, :])
        o_sb = outp.tile([P, D], FP32, tag="o")
        nc.vector.tensor_add(o_sb[:], prod2[:, 10, :], prod2[:, 11, :])

        nc.sync.dma_start(out[tok], o_sb[:])


_NC_CACHE = {}


def build_nc(repeat=1):
    if repeat in _NC_CACHE:
        return _NC_CACHE[repeat]
    nc = bacc.Bacc("TRN2", target_bir_lowering=False, debug=False,
                   num_devices=N_CORES)
    x = nc.dram_tensor("x", [T, D], FP32, kind="ExternalInput").ap()
    kh = nc.dram_tensor("kh", [T, L, D], FP32, kind="ExternalInput").ap()
    wq = nc.dram_tensor("wq", [D, D], FP32, kind="ExternalInput").ap()
    bq = nc.dram_tensor("bq", [D], FP32, kind="ExternalInput").ap()
    ones = nc.dram_tensor("ones", [P], FP32, kind="ExternalInput").ap()
    out = nc.dram_tensor("out", [T, D], FP32, kind="ExternalOutput").ap()
    with tile.TileContext(nc) as tc, ExitStack() as ctx:
        build_body(ctx, tc, out, x, kh, wq, bq, ones, repeat=repeat)
    nc.compile()
    _NC_CACHE[repeat] = nc
    return nc


def make_in_maps(x_current, layer_history, W_q, b_q):
    x_flat = np.ascontiguousarray(
        x_current.reshape(B * S, D), dtype=np.float32)
    k_flat = np.ascontiguousarray(
        layer_history.reshape(B * S, L, D), dtype=np.float32)
    W_q = np.ascontiguousarray(W_q, dtype=np.float32)
    b_q = np.ascontiguousarray(b_q, dtype=np.float32)
    in_maps = []
    for c in range(N_CORES):
        sl = slice(c * T, (c + 1) * T)
        in_maps.append({
            "x": x_flat[sl],
            "kh": k_flat[sl],
            "wq": W_q,
            "bq": b_q,
            "ones": np.ones((P,), np.float32),
        })
    return in_maps


def kernel(x_current, layer_history, W_q, b_q):
    nc = build_nc()
    in_maps = make_in_maps(x_current, layer_history, W_q, b_q)
    res = run_bass_kernel_spmd(nc, in_maps, core_ids=list(range(N_CORES)))
    out = np.concatenate([res.results[c]["out"] for c in range(N_CORES)], axis=0)
    return out.reshape(B, S, D).astype(np.float32)


if __name__ == "__main__":
    rng = np.random.default_rng(0)
    x = rng.standard_normal((B, S, D), dtype=np.float32)
    k = rng.standard_normal((B, S, L, D), dtype=np.float32)
    W = (rng.standard_normal((D, D), dtype=np.float32) / math.sqrt(D)).astype(np.float32)
    b = (rng.standard_normal((D,), dtype=np.float32) * 0.01).astype(np.float32)
    o = kernel(x, k, W, b)
    print("ok", o.shape, o.dtype, float(np.abs(o).mean()))


# revision 15
# speedup vs baseline: 13.8370x; 1.0445x over previous
"""Trainium2 Bass kernel for nn_AttentionResidualBlock.

Computation (per token t, head h):
    q = x @ W_q + b_q                     # [B,S,D]
    scores[t,h,l] = <q[t,h,:], k[t,l,h,:]> / sqrt(hd)
    w = softmax_l(scores)
    out[t,h,:] = sum_l w[t,h,l] * k[t,l,h,:]

Sharding: data-parallel over the 8192 (b,s) tokens -> 8 cores x 1024 tokens.

Per-core layout: token-major.  Each 128-token tile:
  - layer_history loaded as bf16 via SWDGE cast-DMA (HBM traffic unchanged,
    halves SBUF + enables DVE 2x tensor_tensor mode)
  - q_proj on PE in fp32r (full-rate, ~fp32 precision): x-tile transposed via
    PE transpose, then 16 accumulating matmuls + a k=1 "ones" matmul for b_q
  - scores: DVE bf16 mul (q broadcast over l via step-0 AP) + fold-tree
    reduce over hd (bf16 folds 64->8, fp32 tail reduce)
  - softmax over l=12 without max-subtraction (scores ~ N(0,1), exp is safe)
  - weights expanded over hd by exponential doubling (DVE+ACT), then DVE bf16
    mul and fold-tree sum over l (fp32 tail)
"""

import math
from contextlib import ExitStack

import numpy as np

import concourse.bass as bass
import concourse.tile as tile
from concourse import bacc, mybir
from concourse.bass_utils import run_bass_kernel_spmd
from concourse import masks

FP32 = mybir.dt.float32
FP32R = mybir.dt.float32r
BF16 = mybir.dt.bfloat16

B, S, L, D, H = 4, 2048, 12, 1024, 16
HD = D // H
N_CORES = 8
T = B * S // N_CORES          # tokens per core = 1024
P = 128                       # partition tile
NT = T // P                   # 8 token tiles per core
SCALE = 1.0 / math.sqrt(HD)   # 0.125


def _f32r(ap):
    return ap.bitcast(FP32R)


def build_body(ctx, tc, out, x, kh, wq, bq, ones, repeat=1):
    nc = tc.nc

    const_pool = ctx.enter_context(tc.tile_pool(name="const", bufs=1))
    # W as lhsT chunks: w_sb[p, c, j] = W[c*128 + p, j]
    w_sb = const_pool.tile([P, 8, D], FP32R)
    nc.scalar.dma_start(w_sb[:], wq.rearrange("(c p) j -> p c j", p=P).bitcast(FP32R))
    bq_sb = const_pool.tile([1, D], BF16)
    nc.gpsimd.dma_start(bq_sb[:], bq.unsqueeze(0))
    ones_sb = const_pool.tile([1, P], BF16)
    nc.gpsimd.dma_start(ones_sb[:], ones.unsqueeze(0))
    ident = const_pool.tile([P, P], FP32)
    masks.make_identity(nc, ident[:])

    kp = ctx.enter_context(tc.tile_pool(name="k", bufs=2))
    xp = ctx.enter_context(tc.tile_pool(name="x", bufs=2))
    xtp = ctx.enter_context(tc.tile_pool(name="xt", bufs=2))
    qp = ctx.enter_context(tc.tile_pool(name="q", bufs=2))
    prodp = ctx.enter_context(tc.tile_pool(name="prod", bufs=2))
    wbp = ctx.enter_context(tc.tile_pool(name="wb", bufs=2))
    sp = ctx.enter_context(tc.tile_pool(name="smx", bufs=2))
    ps_t = ctx.enter_context(tc.tile_pool(name="ps_t", bufs=2, space="PSUM"))
    ps_q = ctx.enter_context(tc.tile_pool(name="ps_q", bufs=2, space="PSUM"))

    for tt in range(NT * repeat):
        tt = tt % NT
        tok = slice(tt * P, (tt + 1) * P)

        # ---- loads ----
        k_bf = kp.tile([P, L, D], BF16, tag="k")
        nc.gpsimd.dma_start(k_bf[:], kh[tok])  # fp32 -> bf16 cast DMA
        x_sb = xp.tile([P, D], FP32, tag="x")
        nc.sync.dma_start(x_sb[:], x[tok])

        # ---- transpose x tile: xt[p, c, t] = x[tok][t, c*128+p] ----
        xt_sb = xtp.tile([P, 8, P], FP32R, tag="xt")
        for c in range(8):
            xt_ps = ps_t.tile([P, P], FP32, tag="xtps")
            nc.tensor.transpose(xt_ps[:], x_sb[:, c * P:(c + 1) * P], ident[:])
            nc.scalar.copy(xt_sb[:, c, :], xt_ps[:])

        # ---- q = x @ W + b (token-major PSUM [t, d_out]) ----
        q_ps = ps_q.tile([P, D], FP32, tag="qps")
        for half in range(2):
            n0 = half * 512
            for c in range(8):
                nc.tensor.matmul(
                    q_ps[:, n0:n0 + 512],
                    lhsT=xt_sb[:, c, :],
                    rhs=w_sb[:, c, n0:n0 + 512],
                    start=(c == 0),
                    stop=False,
                )
            nc.tensor.matmul(
                q_ps[:, n0:n0 + 512],
                lhsT=ones_sb[:],
                rhs=bq_sb[:, n0:n0 + 512],
                start=False,
                stop=True,
            )
        # q -> SBUF bf16, folding in 1/sqrt(hd)
        q_bf = qp.tile([P, D], BF16, tag="q")
        nc.scalar.mul(q_bf[:], q_ps[:], SCALE)

        # ---- scores: prod = k * q (broadcast over l), fold-reduce over hd ----
        k4 = k_bf[:].rearrange("p l (h e) -> p l h e", h=H)
        qv = (
            q_bf[:]
            .rearrange("p (h e) -> p h e", h=H)
            .unsqueeze(1)
            .broadcast_to([P, L, H, HD])
        )
        prod = prodp.tile([P, L, H, HD], BF16, tag="prod")
        nc.vector.tensor_mul(prod[:], k4, qv)

        # in-place fold tree over hd: 64->32->16->8->4->2, then fp32 tail add.
        # dst aliases in1 exactly (same element positions) which is safe for
        # the streaming DVE.
        off = 0
        for w0 in (32, 16, 8, 4, 2):
            nc.vector.tensor_add(
                prod[:, :, :, off + w0:off + 2 * w0],
                prod[:, :, :, off:off + w0],
                prod[:, :, :, off + w0:off + 2 * w0],
            )
            off += w0
        # off == 62: two surviving partials at 62, 63
        scr = sp.tile([P, L, H], FP32, tag="scr")
        nc.vector.tensor_add(
            scr[:].unsqueeze(3), prod[:, :, :, 62:63], prod[:, :, :, 63:64]
        )

        # ---- softmax over l (no max subtraction) ----
        es = sp.tile([P, L, H], FP32, tag="es")
        nc.scalar.activation(es[:], scr[:], mybir.ActivationFunctionType.Exp)
        den = sp.tile([P, H], FP32, tag="den")
        nc.vector.tensor_reduce(
            den[:],
            es[:].rearrange("p l h -> p h l"),
            axis=mybir.AxisListType.X,
            op=mybir.AluOpType.add,
        )
        rd = sp.tile([P, H], FP32, tag="rd")
        nc.vector.reciprocal(rd[:], den[:])

        # normalized weights into slots 0,1 of the expanded tile (a bf16
        # pair = one fp32 word), then fp32-word doubling broadcast on ACT.
        wb = wbp.tile([P, L, H, HD], BF16, tag="wb")
        rdv = rd[:].unsqueeze(1).broadcast_to([P, L, H]).unsqueeze(3)
        nc.vector.tensor_mul(
            wb[:, :, :, 0:2],
            es[:].unsqueeze(3).broadcast_to([P, L, H, 2]),
            rdv.broadcast_to([P, L, H, 2]),
        )
        # expand each bf16 pair (one fp32 word) across hd with a single
        # step-0-source broadcast copy on ACT, split into l-halves so the
        # first half of the ws-mul overlaps the second half's expansion.
        wbf = wb[:].bitcast(FP32)  # [P, L, H, 32] fp32 words (bf16 pairs)
        prod2 = prodp.tile([P, L, D], BF16, tag="prod")
        wbflat = wb[:].rearrange("p l h e -> p l (h e)")
        for lh in range(2):
            ls = slice(lh * 6, (lh + 1) * 6)
            nc.scalar.copy(
                wbf[:, ls, :, 1:32],
                wbf[:, ls, :, 0:1].broadcast_to([P, 6, H, 31]),
            )
            nc.vector.tensor_mul(
                prod2[:, ls, :], k_bf[:, ls, :], wbflat[:, ls, :]
            )
        # in-place fold over l: 12->6->3, then 3->1 with fp32 tail
        nc.vector.tensor_add(prod2[:, 6:12, :], prod2[:, 0:6, :], prod2[:, 6:12, :])
        nc.vector.tensor_add(prod2[:, 9:12, :], prod2[:, 6:9, :], prod2[:, 9:12, :])
        nc.vector.tensor_add(prod2[:, 10, :], prod2[:, 9, :], prod2[:, 10, :])
        o_sb = xtp.tile([P, D], FP32, tag="xt")
        nc.vector.tensor_add(o_sb[:], prod2[:, 10, :], prod2[:, 11, :])

        nc.sync.dma_start(out[tok], o_sb[:])


_NC_CACHE = {}


def build_nc(repeat=1):
    if repeat in _NC_CACHE:
        return _NC_CACHE[repeat]
    nc = bacc.Bacc("TRN2", target_bir_lowering=False, debug=False,
                   num_devices=N_CORES)
    x = nc.dram_tensor("x", [T, D], FP32, kind="ExternalInput").ap()
    kh = nc.dram_tensor("kh", [T, L, D], FP32, kind="ExternalInput").ap()
    wq = nc.dram_tensor("wq", [D, D], FP32, kind="ExternalInput").ap()
    bq = nc.dram_tensor("bq", [D], FP32, kind="ExternalInput").ap()
    ones = nc.dram_tensor("ones", [P], FP32, kind="ExternalInput").ap()
    out = nc.dram_tensor("out", [T, D], FP32, kind="ExternalOutput").ap()
    with tile.TileContext(nc) as tc, ExitStack() as ctx:
        build_body(ctx, tc, out, x, kh, wq, bq, ones, repeat=repeat)
    nc.compile()
    _NC_CACHE[repeat] = nc
    return nc


def make_in_maps(x_current, layer_history, W_q, b_q):
    x_flat = np.ascontiguousarray(
        x_current.reshape(B * S, D), dtype=np.float32)
    k_flat = np.ascontiguousarray(
        layer_history.reshape(B * S, L, D), dtype=np.float32)
    W_q = np.ascontiguousarray(W_q, dtype=np.float32)
    b_q = np.ascontiguousarray(b_q, dtype=np.float32)
    in_maps = []
    for c in range(N_CORES):
        sl = slice(c * T, (c + 1) * T)
        in_maps.append({
            "x": x_flat[sl],
            "kh": k_flat[sl],
            "wq": W_q,
            "bq": b_q,
            "ones": np.ones((P,), np.float32),
        })
    return in_maps


def kernel(x_current, layer_history, W_q, b_q):
    nc = build_nc()
    in_maps = make_in_maps(x_current, layer_history, W_q, b_q)
    res = run_bass_kernel_spmd(nc, in_maps, core_ids=list(range(N_CORES)))
    out = np.concatenate([res.results[c]["out"] for c in range(N_CORES)], axis=0)
    return out.reshape(B, S, D).astype(np.float32)


if __name__ == "__main__":
    rng = np.random.default_rng(0)
    x = rng.standard_normal((B, S, D), dtype=np.float32)
    k = rng.standard_normal((B, S, L, D), dtype=np.float32)
    W = (rng.standard_normal((D, D), dtype=np.float32) / math.sqrt(D)).astype(np.float32)
    b = (rng.standard_normal((D,), dtype=np.float32) * 0.01).astype(np.float32)
    o = kernel(x, k, W, b)
    print("ok", o.shape, o.dtype, float(np.abs(o).mean()))


# revision 18
# speedup vs baseline: 14.3877x; 1.0398x over previous
"""Trainium2 Bass kernel for nn_AttentionResidualBlock.

Computation (per token t, head h):
    q = x @ W_q + b_q                     # [B,S,D]
    scores[t,h,l] = <q[t,h,:], k[t,l,h,:]> / sqrt(hd)
    w = softmax_l(scores)
    out[t,h,:] = sum_l w[t,h,l] * k[t,l,h,:]

Sharding: data-parallel over the 8192 (b,s) tokens -> 8 cores x 1024 tokens.

Per-core layout: token-major.  Each 128-token tile:
  - layer_history loaded as bf16 via SWDGE cast-DMA (HBM traffic unchanged,
    halves SBUF + enables DVE 2x tensor_tensor mode)
  - q_proj on PE in fp32r (full-rate, ~fp32 precision): x-tile transposed via
    PE transpose, then 16 accumulating matmuls + a k=1 "ones" matmul for b_q
  - scores: DVE bf16 mul (q broadcast over l via step-0 AP) + fold-tree
    reduce over hd (bf16 folds 64->8, fp32 tail reduce)
  - softmax over l=12 without max-subtraction (scores ~ N(0,1), exp is safe)
  - weights expanded over hd by exponential doubling (DVE+ACT), then DVE bf16
    mul and fold-tree sum over l (fp32 tail)
"""

import math
from contextlib import ExitStack

import numpy as np

import concourse.bass as bass
import concourse.tile as tile
from concourse import bacc, mybir
from concourse.bass_utils import run_bass_kernel_spmd
from concourse import masks

FP32 = mybir.dt.float32
FP32R = mybir.dt.float32r
BF16 = mybir.dt.bfloat16

B, S, L, D, H = 4, 2048, 12, 1024, 16
HD = D // H
N_CORES = 8
T = B * S // N_CORES          # tokens per core = 1024
P = 128                       # partition tile
NT = T // P                   # 8 token tiles per core
SCALE = 1.0 / math.sqrt(HD)   # 0.125


def _f32r(ap):
    return ap.bitcast(FP32R)


def build_body(ctx, tc, out, x, kh, wq, bq, ones, repeat=1):
    nc = tc.nc

    const_pool = ctx.enter_context(tc.tile_pool(name="const", bufs=1))
    # W as lhsT chunks: w_sb[p, c, j] = W[c*128 + p, j]
    w_sb = const_pool.tile([P, 8, D], FP32R)
    wqr = wq.rearrange("(c p) j -> p c j", p=P).bitcast(FP32R)
    nc.scalar.dma_start(w_sb[:, :, 0:512], wqr[:, :, 0:512])
    nc.scalar.dma_start(w_sb[:, :, 512:1024], wqr[:, :, 512:1024])
    bq_sb = const_pool.tile([1, D], BF16)
    nc.gpsimd.dma_start(bq_sb[:], bq.unsqueeze(0))
    ones_sb = const_pool.tile([1, P], BF16)
    nc.gpsimd.dma_start(ones_sb[:], ones.unsqueeze(0))
    ident = const_pool.tile([P, P], FP32)
    masks.make_identity(nc, ident[:])

    kp = ctx.enter_context(tc.tile_pool(name="k", bufs=2))
    xp = ctx.enter_context(tc.tile_pool(name="x", bufs=2))
    xtp = ctx.enter_context(tc.tile_pool(name="xt", bufs=2))
    qp = ctx.enter_context(tc.tile_pool(name="q", bufs=2))
    prodp = ctx.enter_context(tc.tile_pool(name="prod", bufs=2))
    wbp = ctx.enter_context(tc.tile_pool(name="wb", bufs=2))
    sp = ctx.enter_context(tc.tile_pool(name="smx", bufs=2))
    ps_t = ctx.enter_context(tc.tile_pool(name="ps_t", bufs=2, space="PSUM"))
    ps_q = ctx.enter_context(tc.tile_pool(name="ps_q", bufs=2, space="PSUM"))

    for tt in range(NT * repeat):
        tt = tt % NT
        tok = slice(tt * P, (tt + 1) * P)

        # ---- loads ----
        k_bf = kp.tile([P, L, D], BF16, tag="k")
        if tt == 0:
            # split the first k load so tile 0's scores can start earlier
            nc.gpsimd.dma_start(k_bf[:, 0:6, :], kh[tok, 0:6, :])
            nc.gpsimd.dma_start(k_bf[:, 6:12, :], kh[tok, 6:12, :])
        else:
            nc.gpsimd.dma_start(k_bf[:], kh[tok])  # fp32 -> bf16 cast DMA
        x_sb = xp.tile([P, D], FP32, tag="x")
        nc.sync.dma_start(x_sb[:], x[tok])

        # ---- transpose x tile: xt[p, c, t] = x[tok][t, c*128+p] ----
        xt_sb = xtp.tile([P, 8, P], FP32R, tag="xt")
        for c in range(8):
            xt_ps = ps_t.tile([P, P], FP32, tag="xtps")
            nc.tensor.transpose(xt_ps[:], x_sb[:, c * P:(c + 1) * P], ident[:])
            nc.scalar.copy(xt_sb[:, c, :], xt_ps[:])

        # ---- q = x @ W + b (token-major PSUM [t, d_out]) ----
        q_ps = ps_q.tile([P, D], FP32, tag="qps")
        for half in range(2):
            n0 = half * 512
            for c in range(8):
                nc.tensor.matmul(
                    q_ps[:, n0:n0 + 512],
                    lhsT=xt_sb[:, c, :],
                    rhs=w_sb[:, c, n0:n0 + 512],
                    start=(c == 0),
                    stop=False,
                )
            nc.tensor.matmul(
                q_ps[:, n0:n0 + 512],
                lhsT=ones_sb[:],
                rhs=bq_sb[:, n0:n0 + 512],
                start=False,
                stop=True,
            )
        # q -> SBUF bf16, folding in 1/sqrt(hd)
        q_bf = qp.tile([P, D], BF16, tag="q")
        nc.scalar.mul(q_bf[:], q_ps[:], SCALE)

        # ---- scores: prod = k * q (broadcast over l), fold-reduce over hd ----
        k4 = k_bf[:].rearrange("p l (h e) -> p l h e", h=H)
        qv = (
            q_bf[:]
            .rearrange("p (h e) -> p h e", h=H)
            .unsqueeze(1)
            .broadcast_to([P, L, H, HD])
        )
        prod = prodp.tile([P, L, H, HD], BF16, tag="prod")
        scr = sp.tile([P, L, H], FP32, tag="scr")
        # tile 0: two l-halves so compute starts as soon as half of k is in
        for ls in ([slice(0, 6), slice(6, 12)] if tt == 0 else [slice(0, L)]):
            nl = ls.stop - ls.start
            nc.vector.tensor_mul(prod[:, ls], k4[:, ls], qv[:, ls])
            # in-place fold tree over hd: 64->32->...->2, then fp32 tail add.
            # dst aliases in1 exactly (same element positions) which is safe
            # for the streaming DVE.
            off = 0
            for w0 in (32, 16, 8, 4, 2):
                nc.vector.tensor_add(
                    prod[:, ls, :, off + w0:off + 2 * w0],
                    prod[:, ls, :, off:off + w0],
                    prod[:, ls, :, off + w0:off + 2 * w0],
                )
                off += w0
            # off == 62: two surviving partials at 62, 63
            nc.vector.tensor_add(
                scr[:, ls].unsqueeze(3),
                prod[:, ls, :, 62:63],
                prod[:, ls, :, 63:64],
            )

        # ---- softmax over l (no max subtraction) ----
        es = sp.tile([P, L, H], FP32, tag="es")
        nc.scalar.activation(es[:], scr[:], mybir.ActivationFunctionType.Exp)
        den = sp.tile([P, H], FP32, tag="den")
        nc.vector.tensor_reduce(
            den[:],
            es[:].rearrange("p l h -> p h l"),
            axis=mybir.AxisListType.X,
            op=mybir.AluOpType.add,
        )
        rd = sp.tile([P, H], FP32, tag="rd")
        nc.vector.reciprocal(rd[:], den[:])

        # normalized weights into slots 0,1 of the expanded tile (a bf16
        # pair = one fp32 word), then fp32-word doubling broadcast on ACT.
        wb = wbp.tile([P, L, H, HD], BF16, tag="wb")
        rdv = rd[:].unsqueeze(1).broadcast_to([P, L, H]).unsqueeze(3)
        nc.vector.tensor_mul(
            wb[:, :, :, 0:2],
            es[:].unsqueeze(3).broadcast_to([P, L, H, 2]),
            rdv.broadcast_to([P, L, H, 2]),
        )
        # expand each bf16 pair (one fp32 word) across hd with a single
        # step-0-source broadcast copy on ACT, split into l-halves so the
        # first half of the ws-mul overlaps the second half's expansion.
        wbf = wb[:].bitcast(FP32)  # [P, L, H, 32] fp32 words (bf16 pairs)
        prod2 = prodp.tile([P, L, D], BF16, tag="prod")
        wbflat = wb[:].rearrange("p l h e -> p l (h e)")
        for lh in range(2):
            ls = slice(lh * 6, (lh + 1) * 6)
            nc.scalar.copy(
                wbf[:, ls, :, 1:32],
                wbf[:, ls, :, 0:1].broadcast_to([P, 6, H, 31]),
            )
            nc.vector.tensor_mul(
                prod2[:, ls, :], k_bf[:, ls, :], wbflat[:, ls, :]
            )
        # in-place fold over l: 12->6->3, then 3->1 with fp32 tail
        nc.vector.tensor_add(prod2[:, 6:12, :], prod2[:, 0:6, :], prod2[:, 6:12, :])
        nc.vector.tensor_add(prod2[:, 9:12, :], prod2[:, 6:9, :], prod2[:, 9:12, :])
        nc.vector.tensor_add(prod2[:, 10, :], prod2[:, 9, :], prod2[:, 10, :])
        o_sb = xtp.tile([P, D], FP32, tag="xt")
        nc.vector.tensor_add(o_sb[:], prod2[:, 10, :], prod2[:, 11, :])

        nc.sync.dma_start(out[tok], o_sb[:])


_NC_CACHE = {}


def build_nc(repeat=1):
    if repeat in _NC_CACHE:
        return _NC_CACHE[repeat]
    nc = bacc.Bacc("TRN2", target_bir_lowering=False, debug=False,
                   num_devices=N_CORES)
    x = nc.dram_tensor("x", [T, D], FP32, kind="ExternalInput").ap()
    kh = nc.dram_tensor("kh", [T, L, D], FP32, kind="ExternalInput").ap()
    wq = nc.dram_tensor("wq", [D, D], FP32, kind="ExternalInput").ap()
    bq = nc.dram_tensor("bq", [D], FP32, kind="ExternalInput").ap()
    ones = nc.dram_tensor("ones", [P], FP32, kind="ExternalInput").ap()
    out = nc.dram_tensor("out", [T, D], FP32, kind="ExternalOutput").ap()
    with tile.TileContext(nc) as tc, ExitStack() as ctx:
        build_body(ctx, tc, out, x, kh, wq, bq, ones, repeat=repeat)
    nc.compile()
    _NC_CACHE[repeat] = nc
    return nc


def make_in_maps(x_current, layer_history, W_q, b_q):
    x_flat = np.ascontiguousarray(
        x_current.reshape(B * S, D), dtype=np.float32)
    k_flat = np.ascontiguousarray(
        layer_history.reshape(B * S, L, D), dtype=np.float32)
    W_q = np.ascontiguousarray(W_q, dtype=np.float32)
    b_q = np.ascontiguousarray(b_q, dtype=np.float32)
    in_maps = []
    for c in range(N_CORES):
        sl = slice(c * T, (c + 1) * T)
        in_maps.append({
            "x": x_flat[sl],
            "kh": k_flat[sl],
            "wq": W_q,
            "bq": b_q,
            "ones": np.ones((P,), np.float32),
        })
    return in_maps


def kernel(x_current, layer_history, W_q, b_q):
    nc = build_nc()
    in_maps = make_in_maps(x_current, layer_history, W_q, b_q)
    res = run_bass_kernel_spmd(nc, in_maps, core_ids=list(range(N_CORES)))
    out = np.concatenate([res.results[c]["out"] for c in range(N_CORES)], axis=0)
    return out.reshape(B, S, D).astype(np.float32)


if __name__ == "__main__":
    rng = np.random.default_rng(0)
    x = rng.standard_normal((B, S, D), dtype=np.float32)
    k = rng.standard_normal((B, S, L, D), dtype=np.float32)
    W = (rng.standard_normal((D, D), dtype=np.float32) / math.sqrt(D)).astype(np.float32)
    b = (rng.standard_normal((D,), dtype=np.float32) * 0.01).astype(np.float32)
    o = kernel(x, k, W, b)
    print("ok", o.shape, o.dtype, float(np.abs(o).mean()))
